# revision 3
# baseline (speedup 1.0000x reference)
"""Distributed GQA attention kernel for 8 TRN2 NeuronCores — v2.

Strategy (tensor-parallel over heads, A2A re-shard before o_proj), same
algorithm as v1 but restructured for one continuous PE stream:

  - QKV projection streams k-tiles as the hT DMA lands (kt-outer,
    n-inner over 7 PSUM banks: kv + q-pair-0 chunks 0-2), so the PE
    starts ~6us in instead of waiting for the full 8MB hT.
  - RoPE (neox) reads the qkv PSUM banks directly (no staging copies);
    the "swap" is folded into a host-permuted sin table so rope is
    2 muls + 4 block adds per chunk, mostly bf16 2x DVE ops.
  - Causal mask applied by PRE-INITIALIZING the diagonal 128-col PSUM
    stripe with -400 via a tiny triangle matmul, then accumulating the
    scores on top (start=False).  Kills the per-block mask multiply on
    Vector entirely.  Exp is trimmed to skip fully-masked columns, the
    skipped e columns are zeroed by gpsimd memsets.
  - Attention is software-pipelined (scores kb, PV kb-1) and the PE
    gaps left by the Exp-paced chain are filled with interleaved
    "filler" matmuls: remaining q projections (pair-0 n3 + all of
    pair-1), v transposes, and later the even-kt half of o_proj.
  - Two AllToAlls (one per head pair); pair-0's A2A overlaps pair-1's
    attention; o_proj even-kt chunks run during pair-1 + A2A-1, odd-kt
    chunks after A2A-1.  o_proj partials staged in SBUF f32 so PSUM
    pressure stays at 2 banks.
  - DMA issue spread across the SP/ACT/DVE/Pool sequencers (descriptor
    generation is ~0.6-1us per DMA on one sequencer).
  - Output written bf16, host casts to f32.
"""

import os
import numpy as np
import ml_dtypes

import concourse.bass as bass
import concourse.mybir as mybir
from concourse import bacc, tile
import bass_rust as _br

F32 = mybir.dt.float32
BF16 = mybir.dt.bfloat16
BF16_NP = ml_dtypes.bfloat16

# Problem constants (hardcoded per harness contract)
H = 2048
S = 2048
NH = 32
NKV = 8
HD = 64
Q_SIZE = NH * HD      # 2048
KV_SIZE = NKV * HD    # 512
NCORES = 8
QH = NH // NCORES     # 4 q heads per core
ROPE_THETA = 10000.0

P = 128
KT = H // P           # 16 contraction tiles over H
NQ = 512              # query chunk (matmul moving free dim)
NJC = S // NQ         # 4 query chunks
NKB = S // P          # 16 key tiles
SROWS = S // NCORES   # 256 seq rows per core after A2A
MASKVAL = -400.0      # pre-exp mask bias; *0.125 = -50 under exp
USE_MASKMUL = False   # bisect flag: baseline post-exp mask multiply

_NC_CACHE = None
LAST_RESULTS = None


def _build_nc():
    nc = bacc.Bacc(
        "TRN2",
        target_bir_lowering=False,
        debug=False,
        num_devices=NCORES,
    )

    # ---- I/O ----
    hT_d = nc.dram_tensor("hT", [P, KT * S], BF16, kind="ExternalInput")
    wq_d = nc.dram_tensor("wqkvT", [P, KT * 384], BF16, kind="ExternalInput")
    wo_d = nc.dram_tensor("woT", [P, KT * H], BF16, kind="ExternalInput")
    ropec_d = nc.dram_tensor("ropec", [P, S], BF16, kind="ExternalInput")
    ropes_d = nc.dram_tensor("ropesP", [P, S], BF16, kind="ExternalInput")
    identf_d = nc.dram_tensor("identf", [P, P], F32, kind="ExternalInput")
    tri_d = nc.dram_tensor("tri", [P, P], BF16, kind="ExternalInput")
    identb_d = nc.dram_tensor("identb", [P, P], BF16, kind="ExternalInput")
    masks_d = nc.dram_tensor("masks", [P, 4 * 2 * NQ], BF16, kind="ExternalInput")
    out_d = nc.dram_tensor("out", [SROWS, H], F32, kind="ExternalOutput")

    rg = [list(range(NCORES))]

    with tile.TileContext(nc) as tc:
        with (
            tc.tile_pool(name="dram", bufs=1, space="DRAM") as dram,
            tc.tile_pool(name="const", bufs=1) as const,
            tc.tile_pool(name="qk", bufs=1) as qkpool,
            tc.tile_pool(name="esb", bufs=6) as esb,
            tc.tile_pool(name="small", bufs=3) as small,
            tc.tile_pool(name="outp", bufs=2) as outp,
            tc.tile_pool(name="ropet", bufs=2) as ropet,
        ):
            cc_in = [
                dram.tile([NCORES * P, SROWS], BF16, tag=f"cc_in{p}", name=f"cc_in{p}")
                for p in range(2)
            ]
            cc_out = [
                dram.tile([NCORES * P, SROWS], BF16, tag=f"cc_out{p}", name=f"cc_out{p}")
                for p in range(2)
            ]

            # tiny warm-up collective absorbs the ncfw first-collective delay
            ccw_in = dram.tile([NCORES, 16], BF16, tag="ccw_in")
            ccw_out = dram.tile([NCORES, 16], BF16, tag="ccw_out")
            warm_sb = const.tile([NCORES, 16], BF16, tag="warm_sb")
            nc.vector.memset(warm_sb[:], 0.0)
            nc.sync.dma_start(ccw_in[:], warm_sb[:])
            nc.gpsimd.collective_compute(
                "AllToAll",
                mybir.AluOpType.bypass,
                replica_groups=rg,
                ins=[ccw_in.opt()],
                outs=[ccw_out.opt()],
            )

            # ---- persistent SBUF ----
            qpair = [
                qkpool.tile([P, S], BF16, tag=f"qpair{m}", name=f"qpair{m}")
                for m in range(2)
            ]
            kdup = qkpool.tile([P, S], BF16, tag="kdup")
            kvb = qkpool.tile([P, S], BF16, tag="kvb")
            v_aug = qkpool.tile([P, NKB * 128], BF16, tag="v_aug")
            cc_sb = qkpool.tile([P, KT * SROWS], BF16, tag="cc_sb")
            o_even = qkpool.tile([P, 8 * NQ], F32, tag="o_even")

            # ---- DMA issue: weights+hidden first, spread across sequencers ----
            hq_ctx = tc.tile_pool(name="hq", bufs=1)
            hq = hq_ctx.__enter__()
            wq_sb = hq.tile([P, KT * 384], BF16, tag="wq_sb")
            h_tiles = []
            for kt in range(KT):
                nc.sync.dma_start(
                    wq_sb[:, kt * 384 : (kt + 1) * 384],
                    wq_d[:, kt * 384 : (kt + 1) * 384],
                )
                ht = hq.tile([P, S], BF16, tag=f"h{kt}", name=f"h{kt}")
                # two half-tile sub-DMAs on different sequencers so tiles
                # land incrementally across queues
                nc.sync.dma_start(
                    ht[:, 0 : S // 2], hT_d[:, kt * S : kt * S + S // 2]
                )
                nc.sync.dma_start(
                    ht[:, S // 2 : S], hT_d[:, kt * S + S // 2 : (kt + 1) * S]
                )
                h_tiles.append(ht)
            ropec = const.tile([P, S], BF16, tag="ropec")
            nc.sync.dma_start(ropec[:], ropec_d[:])
            ropesP = const.tile([P, S], BF16, tag="ropesP")
            nc.sync.dma_start(ropesP[:], ropes_d[:])
            identf = const.tile([P, P], F32, tag="identf")
            nc.sync.dma_start(identf[:], identf_d[:])
            identb = const.tile([P, P], BF16, tag="identb")
            nc.sync.dma_start(identb[:], identb_d[:])
            tri = const.tile([P, P], BF16, tag="tri")
            nc.sync.dma_start(tri[:], tri_d[:])
            masks = None
            if USE_MASKMUL:
                masks = const.tile([P, 4 * 2 * NQ], BF16, tag="masks")
                nc.sync.dma_start(masks[:], masks_d[:])

            # fill v_aug with ones once; the v DMA-transposes then overwrite
            # the 64-wide value blocks, leaving each 65th column at 1.0
            nc.gpsimd.memset(v_aug[:], 1.0)

            # ================= Phase 1: streaming QKV =================
            # kv group (m=2) all 4 chunks + q-pair-0 (m=0) chunks 0..2,
            # kt-outer so the PE consumes h tiles as they arrive.
            def rope_chunk(dst, dst_rows, src, n, src_is_chunk=False):
                """RoPE from an SBUF staging copy into dst[:, n*NQ:(n+1)*NQ].
                src is a full-width staging tile (k/v) unless src_is_chunk."""
                c0, c1 = NQ * n, NQ * (n + 1)
                s = src[:, 0:NQ] if src_is_chunk else src[:, c0:c1]
                pc = ropet.tile([P, NQ], BF16, tag="pc", name="pc")
                nc.vector.tensor_mul(
                    pc[0:dst_rows, :], s[0:dst_rows, :], ropec[0:dst_rows, c0:c1]
                )
                swp = ropet.tile([P, NQ], BF16, tag="swp", name="swp")
                for b in range(dst_rows // 64):
                    o = 64 * b
                    nc.vector.tensor_copy(swp[o : o + 32, :], s[o + 32 : o + 64, :])
                    nc.vector.tensor_copy(swp[o + 32 : o + 64, :], s[o : o + 32, :])
                psn = ropet.tile([P, NQ], BF16, tag="psn", name="psn")
                nc.vector.tensor_mul(
                    psn[0:dst_rows, :], swp[0:dst_rows, :], ropesP[0:dst_rows, c0:c1]
                )
                nc.vector.tensor_add(
                    dst[0:dst_rows, c0:c1], pc[0:dst_rows, :], psn[0:dst_rows, :]
                )
                return pc, psn

            ph1_ctx = tc.tile_pool(name="ps1", bufs=1, space="PSUM")
            ps1 = ph1_ctx.__enter__()
            kv_ps = [ps1.tile([P, NQ], F32, tag=f"kv{n}", name=f"kv{n}") for n in range(4)]
            m0_ps = [ps1.tile([P, NQ], F32, tag=f"m0{n}", name=f"m0{n}") for n in range(3)]
            for kt in range(KT):
                for n in range(4):
                    nc.tensor.matmul(
                        kv_ps[n][:],
                        wq_sb[:, kt * 384 + 256 : kt * 384 + 384],
                        h_tiles[kt][:, NQ * n : NQ * (n + 1)],
                        start=(kt == 0),
                        stop=(kt == KT - 1),
                    )
                for n in range(3):
                    nc.tensor.matmul(
                        m0_ps[n][:],
                        wq_sb[:, kt * 384 : kt * 384 + 128],
                        h_tiles[kt][:, NQ * n : NQ * (n + 1)],
                        start=(kt == 0),
                        stop=(kt == KT - 1),
                    )

            # kv/q consumption, deadline order: chunk n gates attention jc=n.
            # ACT: v copies (it is idle until the first exp); DVE: k/q rope
            # direct from PSUM; k row duplication on ACT; v transposed into
            # v_aug by DMA-transpose (XBAR) - no PE or PSUM involved.
            q0stg = [
                qkpool.tile([P, NQ], BF16, tag=f"q0stg{n}", name=f"q0stg{n}")
                for n in range(3)
            ]
            # stage all 7 banks to SBUF up front so the attention pools can
            # allocate PSUM immediately; ropes then run from SBUF, overlapped
            # with attention
            for n in range(4):
                nc.vector.tensor_copy(
                    kvb[:, NQ * n : NQ * (n + 1)], kv_ps[n][:]
                )
                if n < 3:
                    nc.vector.tensor_copy(q0stg[n][:], m0_ps[n][:])
            for n in range(4):
                pc, psn = rope_chunk(kdup, 64, kvb, n)
                nc.vector.tensor_add(
                    kdup[64:128, NQ * n : NQ * (n + 1)], pc[0:64, :], psn[0:64, :]
                )
                if n < 3:
                    rope_chunk(qpair[0], 128, q0stg[n], n, src_is_chunk=True)
            # jc0's v transposes (kb 0-3) in the spare phase-1 bank
            vtps = ps1.tile([P, NKB * 64], BF16, tag="vtps", name="vtps")
            for kb in range(NKB):
                nc.tensor.transpose(
                    vtps[:, 64 * kb : 64 * (kb + 1)],
                    kvb[64:128, P * kb : P * (kb + 1)],
                    identb[64:128, 64:128],
                )
            nc.vector.tensor_copy(
                v_aug[:].rearrange("p (kb c) -> p kb c", kb=NKB)[:, :, 0:64],
                vtps[:].rearrange("p (kb c) -> p kb c", kb=NKB),
            )
            # cols 64-127 of each block stay 1.0: the PV matmul replicates the
            # softmax denominator into PSUM rows 64-127 (no broadcast needed)
            ph1_ctx.__exit__(None, None, None)

            # w_o streamed at the pair-0/pair-1 boundary (SBUF freed by h)
            wo_ctx = None
            wo_tiles = []

            def emit_wo_stream():
                nonlocal wo_ctx
                wo_ctx = tc.tile_pool(name="wo_stream", bufs=4)
                wo_stream = wo_ctx.__enter__()
                for n in range(NJC):
                    wos = wo_stream.tile(
                        [P, KT * NQ], BF16, tag="wos", name=f"wos{n}"
                    )
                    for kg in range(4):
                        nc.sync.dma_start(
                            wos[:].rearrange("p (kt c) -> p kt c", kt=KT)[
                                :, 4 * kg : 4 * (kg + 1), :
                            ],
                            wo_d[:].rearrange("p (kt c) -> p kt c", kt=KT)[
                                :, 4 * kg : 4 * (kg + 1), NQ * n : NQ * (n + 1)
                            ],
                        )
                    wo_tiles.append(wos)

            # ============ Phase 2: attention with filler interleave ============
            ps_s_ctx = tc.tile_pool(name="ps_s", bufs=2, space="PSUM")
            ps_s = ps_s_ctx.__enter__()
            ps_ctx_ctx = tc.tile_pool(name="ps_ctx", bufs=1, space="PSUM")
            ps_ctx = ps_ctx_ctx.__enter__()
            fil_ctx = tc.tile_pool(name="fil", bufs=2, space="PSUM")
            fil = fil_ctx.__enter__()

            # ---- filler op queue: list of closures, each emits 1 PE op ----
            filler_q = []
            fil_state = {"tile": None, "count": 0, "kind": None, "n": None}

            def emit_qproj_mm(dst_pair, m_off, n, kt, fil_tile):
                nc.tensor.matmul(
                    fil_tile[:],
                    wq_sb[:, kt * 384 + m_off : kt * 384 + m_off + 128],
                    h_tiles[kt][:, NQ * n : NQ * (n + 1)],
                    start=(kt == 0),
                    stop=(kt == KT - 1),
                )

            def make_qproj_chunk(dst_pair, m_off, n):
                # 16 matmuls accumulating into one fil tile, then rope
                ops = []
                holder = {}

                def first(kt=0):
                    holder["t"] = fil.tile([P, NQ], F32, tag="fil", name=f"qp{dst_pair}{n}")
                    emit_qproj_mm(dst_pair, m_off, n, 0, holder["t"])
                ops.append(first)
                for kt in range(1, KT):
                    ops.append(lambda kt=kt: emit_qproj_mm(
                        dst_pair, m_off, n, kt, holder["t"]))

                def last():
                    stg = ropet.tile([P, NQ], BF16, tag="stg", name="stg")
                    nc.vector.tensor_copy(stg[:], holder["t"][:])
                    rope_chunk(qpair[dst_pair], 128, stg, n, src_is_chunk=True)
                ops.append(last)   # DVE ops ride along with the last PE op
                return ops

            def make_oproj_chunk(c, parity):
                # 8 matmuls (kts of one parity) for output chunk c=(n,m)
                n, m = c // 2, c % 2
                ops = []
                holder = {}

                def mm(kk, first):
                    if first:
                        holder["t"] = fil.tile([P, NQ], F32, tag="fil", name=f"op{parity}{c}")
                    kt = 2 * kk + parity
                    nc.tensor.matmul(
                        holder["t"][:],
                        cc_sb[:, kt * SROWS + P * m : kt * SROWS + P * (m + 1)],
                        wo_tiles[n][:, kt * NQ : (kt + 1) * NQ],
                        start=(kk == 0),
                        stop=(kk == 7),
                    )
                ops.append(lambda: mm(0, True))
                for kk in range(1, 8):
                    ops.append(lambda kk=kk: mm(kk, False))

                if parity == 0:
                    def last():
                        nc.vector.tensor_copy(
                            o_even[:, NQ * c : NQ * (c + 1)], holder["t"][:]
                        )
                    ops.append(last)
                else:
                    def last():
                        ot = outp.tile([P, NQ], F32, tag="ot")
                        nc.vector.tensor_add(
                            ot[:], o_even[:, NQ * c : NQ * (c + 1)], holder["t"][:]
                        )
                        nc.sync.dma_start(
                            out_d[P * m : P * (m + 1), NQ * n : NQ * (n + 1)], ot[:]
                        )
                    ops.append(last)
                return ops

            # pair-0 fillers, deadline order: v transposes for jc1-3, then
            # q-proj pair0-n3, then all of pair-1's q projections
            filler_q.append(make_qproj_chunk(0, 0, 3))
            for n in range(4):
                filler_q.append(make_qproj_chunk(1, 128, n))
            # flatten with chunk boundaries preserved (chunks already ordered
            # by deadline; ops within a chunk must stay in order)
            flat_fillers = [op for ch in filler_q for op in ch]
            fill_pos = {"i": 0}

            def emit_fillers(k):
                for _ in range(k):
                    if fill_pos["i"] >= len(flat_fillers):
                        return
                    flat_fillers[fill_pos["i"]]()
                    fill_pos["i"] += 1

            last_scatter = [None, None]
            early_p1_scatter = [None]
            cc_insts = []
            for p in range(2):
                if p == 1:
                    # drain leftover q-proj fillers (they read h), free the
                    # h/wq SBUF, then start streaming w_o into that space
                    emit_fillers(len(flat_fillers) - fill_pos["i"])
                    hq_ctx.__exit__(None, None, None)
                    emit_wo_stream()
                for jc in range(NJC):
                    nkb = 4 * (jc + 1)
                    ctxs = [
                        ps_ctx.tile([P, NQ], F32, tag=f"ctx{hh}", name=f"ctx{hh}")
                        for hh in range(2)
                    ]
                    sp_prev = None
                    e_prev = None
                    for kb in range(nkb):
                        d = kb - 4 * jc
                        emit_fillers(2 if p == 0 else 3)
                        sp = ps_s.tile([P, 2 * NQ], F32, tag="sp", name="sp")
                        if d >= 0 and USE_MASKMUL:
                            for hh in range(2):
                                base = 64 * hh
                                nc.tensor.matmul(
                                    sp[:, NQ * hh : NQ * (hh + 1)],
                                    kdup[base : base + 64, P * kb : P * (kb + 1)],
                                    qpair[p][base : base + 64, NQ * jc : NQ * (jc + 1)],
                                    start=True,
                                    stop=True,
                                )
                        elif d >= 0:
                            for hh in range(2):
                                co = NQ * hh + 128 * d
                                nc.tensor.matmul(
                                    sp[:, co : co + 128],
                                    identb[:],
                                    tri[:],
                                    start=True,
                                    stop=False,
                                    skip_group_check=True,
                                )
                            for hh in range(2):
                                base = 64 * hh
                                co = NQ * hh + 128 * d
                                nc.tensor.matmul(
                                    sp[:, co : co + 128],
                                    kdup[base : base + 64, P * kb : P * (kb + 1)],
                                    qpair[p][
                                        base : base + 64,
                                        NQ * jc + 128 * d : NQ * jc + 128 * (d + 1),
                                    ],
                                    start=False,
                                    stop=True,
                                    skip_group_check=True,
                                )
                            if d < 3:
                                for hh in range(2):
                                    base = 64 * hh
                                    co = NQ * hh + 128 * (d + 1)
                                    nc.tensor.matmul(
                                        sp[:, co : NQ * (hh + 1)],
                                        kdup[base : base + 64, P * kb : P * (kb + 1)],
                                        qpair[p][
                                            base : base + 64,
                                            NQ * jc + 128 * (d + 1) : NQ * (jc + 1),
                                        ],
                                        start=True,
                                        stop=True,
                                        skip_group_check=True,
                                    )
                        else:
                            for hh in range(2):
                                base = 64 * hh
                                nc.tensor.matmul(
                                    sp[:, NQ * hh : NQ * (hh + 1)],
                                    kdup[base : base + 64, P * kb : P * (kb + 1)],
                                    qpair[p][base : base + 64, NQ * jc : NQ * (jc + 1)],
                                    start=True,
                                    stop=True,
                                )
                        # PV for previous kb (software pipeline depth 1);
                        # diagonal blocks contribute nothing to the columns
                        # left of their stripe, so trim both e and ctx there
                        if kb > 0:
                            off = 128 * (d - 1) if (d >= 1 and not USE_MASKMUL) else 0
                            for hh in range(2):
                                nc.tensor.matmul(
                                    ctxs[hh][:, off:NQ],
                                    v_aug[:, (kb - 1) * 128 : kb * 128],
                                    e_prev[:, NQ * hh + off : NQ * (hh + 1)],
                                    start=(kb - 1 == 0),
                                    stop=False,
                                    skip_group_check=True,
                                )
                        # exp, trimmed to the live columns (per-hh 2D APs)
                        e = esb.tile([P, 2 * NQ], BF16, tag="e", name="e")
                        if USE_MASKMUL:
                            nc.scalar.activation(
                                e[:], sp[:], mybir.ActivationFunctionType.Exp,
                                scale=0.125,
                            )
                            if d >= 0:
                                nc.vector.tensor_mul(
                                    e[:], e[:],
                                    masks[:, 2 * NQ * d : 2 * NQ * (d + 1)],
                                )
                        elif d >= 1:
                            for hh in range(2):
                                nc.scalar.activation(
                                    e[:, NQ * hh + 128 * d : NQ * (hh + 1)],
                                    sp[:, NQ * hh + 128 * d : NQ * (hh + 1)],
                                    mybir.ActivationFunctionType.Exp,
                                    scale=0.125,
                                )
                        elif not USE_MASKMUL:
                            nc.scalar.activation(
                                e[:], sp[:], mybir.ActivationFunctionType.Exp,
                                scale=0.125,
                            )
                        sp_prev, e_prev = sp, e
                    # final PV (always a d=3 diagonal block)
                    foff = 0 if USE_MASKMUL else 384
                    for hh in range(2):
                        nc.tensor.matmul(
                            ctxs[hh][:, foff:NQ],
                            v_aug[:, (nkb - 1) * 128 : nkb * 128],
                            e_prev[:, NQ * hh + foff : NQ * (hh + 1)],
                            start=False,
                            stop=True,
                            skip_group_check=True,
                        )
                    # epilogue: normalize + scatter
                    for hh in range(2):
                        cb = small.tile([P, NQ], F32, tag="cb")
                        nc.vector.tensor_copy(cb[:], ctxs[hh][:, :])
                        # denominators sit replicated in rows 64-127; stage at
                        # base 0 (custom DVE ops misread at nonzero base)
                        rb = small.tile([64, NQ], F32, tag="rb")
                        nc.vector.tensor_copy(rb[:], cb[64:128, :])
                        rec = small.tile([64, NQ], F32, tag="rec")
                        nc.vector.reciprocal_approx_fast(rec[:], rb[:])
                        ctxn = small.tile([64, NQ], BF16, tag="ctxn")
                        nc.vector.tensor_mul(ctxn[:], cb[0:64, :], rec[:])
                        for half in range(2):
                            j = 2 * jc + half
                            last_scatter[p] = nc.sync.dma_start(
                                cc_in[p][
                                    P * j + 64 * hh : P * j + 64 * (hh + 1), :
                                ],
                                ctxn[:, SROWS * half : SROWS * (half + 1)],
                            )
                            if p == 1 and jc == 0:
                                early_p1_scatter[0] = last_scatter[p]
                # A2A for this pair
                cc_insts.append(
                    nc.gpsimd.collective_compute(
                        "AllToAll",
                        mybir.AluOpType.bypass,
                        replica_groups=rg,
                        ins=[cc_in[p].opt()],
                        outs=[cc_out[p].opt()],
                    )
                )

            # ---- tail (baseline structure): drain fillers, then cc_sb loads
            # for both parities; each waits on its own A2A, so even-kt o_proj
            # chunks execute during A2A-1 ----
            emit_fillers(len(flat_fillers) - fill_pos["i"])
            # even loads first: odd loads wait on A2A-1 and would otherwise
            # head-of-line block the even data the A2A-1-covering even o_proj
            # pass needs
            for par in range(2):
                for j in range(NCORES):
                    dma = nc.sync.dma_start(
                        cc_sb[
                            :,
                            (2 * j + par) * SROWS : (2 * j + par) * SROWS + SROWS,
                        ],
                        cc_out[par][j * P : (j + 1) * P, :],
                    )
                    anchor = early_p1_scatter[0] if par == 0 else last_scatter[1]
                    _br.add_dep_helper(
                        dma.ins, anchor.ins, sync=True,
                        reason="cc_sb read ordered behind scatter traffic",
                    )
            for c in range(8):
                for op in make_oproj_chunk(c, 0):
                    op()
            # odd passes + combine + output
            for c in range(8):
                for op in make_oproj_chunk(c, 1):
                    op()

            fil_ctx.__exit__(None, None, None)
            ps_ctx_ctx.__exit__(None, None, None)
            ps_s_ctx.__exit__(None, None, None)
            wo_ctx.__exit__(None, None, None)

    nc.compile()
    return nc


def _get_nc():
    global _NC_CACHE
    if _NC_CACHE is None:
        _NC_CACHE = _build_nc()
    return _NC_CACHE


def _stage_inputs(position_ids, hidden_states, w_qkv, w_o):
    """Host-side sharding / layout staging. Returns in_maps for 8 cores."""
    pos = np.asarray(position_ids)[0].astype(np.float32)            # [S]
    hidden = np.asarray(hidden_states, dtype=np.float32)[0]         # [S, H]
    w_qkv = np.asarray(w_qkv, dtype=np.float32)                     # [3072, H]
    w_o = np.asarray(w_o, dtype=np.float32)                         # [H, Q_SIZE]

    # hT tiles: [H, S] -> [128, KT*S] (k-tile kt at cols kt*S..)
    hT = np.ascontiguousarray(hidden.T)
    hT_r = np.ascontiguousarray(
        hT.reshape(KT, P, S).transpose(1, 0, 2).reshape(P, KT * S)
    ).astype(BF16_NP)

    # w_o.T tiles: [Q_SIZE, H] -> [128, KT*H]
    woT = np.ascontiguousarray(w_o.T)
    woT_r = np.ascontiguousarray(
        woT.reshape(KT, P, H).transpose(1, 0, 2).reshape(P, KT * H)
    ).astype(BF16_NP)

    # rope tables in [d, s] layout for a [128 = 2 heads x 64] tile; the sin
    # product reads its x input at the partner rows, so the table rows hold
    # the sign for the OUTPUT row: [-sin; +sin] per 64-row head block
    inv_freq = (1.0 / (ROPE_THETA ** (np.arange(0, HD, 2, dtype=np.float32) / HD)))
    ang = pos[:, None] * inv_freq[None, :]                          # [S, 32]
    cosT = np.cos(ang).T.astype(np.float32)                         # [32, S]
    sinT = np.sin(ang).T.astype(np.float32)
    ropec = np.concatenate([cosT, cosT, cosT, cosT], axis=0)        # [128, S]
    ropesP = np.concatenate([-sinT, sinT, -sinT, sinT], axis=0)     # [128, S]

    ident = np.eye(P, dtype=np.float32)
    f = np.arange(NQ)
    mask_list = []
    for d in range(4):
        pp = np.arange(P)[:, None] + 128 * d
        mk = (pp <= f[None, :]).astype(BF16_NP)
        mask_list.append(mk)
        mask_list.append(mk)
    masks_np = np.concatenate(mask_list, axis=1)                    # [128, 8*NQ]
    tri = np.where(
        np.arange(P)[:, None] > np.arange(P)[None, :], MASKVAL, 0.0
    ).astype(np.float32)

    in_maps = []
    for i in range(NCORES):
        rows_q = w_qkv[QH * HD * i : QH * HD * (i + 1)]             # [256, H]
        row_k = w_qkv[Q_SIZE + HD * i : Q_SIZE + HD * (i + 1)]      # [64, H]
        row_v = w_qkv[Q_SIZE + KV_SIZE + HD * i : Q_SIZE + KV_SIZE + HD * (i + 1)]
        wshard = np.concatenate([rows_q, row_k, row_v], axis=0)     # [384, H]
        wqkvT = np.ascontiguousarray(wshard.T)                      # [H, 384]
        wqkvT_r = np.ascontiguousarray(
            wqkvT.reshape(KT, P, 384).transpose(1, 0, 2).reshape(P, KT * 384)
        ).astype(BF16_NP)
        in_maps.append(
            {
                "hT": hT_r,
                "wqkvT": wqkvT_r,
                "woT": woT_r,
                "ropec": ropec.astype(BF16_NP),
                "ropesP": ropesP.astype(BF16_NP),
                "identf": ident,
                "identb": ident.astype(BF16_NP),
                "tri": tri.astype(BF16_NP),
                "masks": masks_np,
            }
        )
    return in_maps


def _ensure_ntff_hook():
    """The container's antenv stub lacks axon_hooks, so trn_boot silently
    skipped NTFF hook registration. Recreate the module and register the
    ctypes-based hook so run_bass_kernel_spmd(trace=True) can profile."""
    import sys
    import types

    if "antenv.axon_hooks" in sys.modules:
        return
    try:
        import antenv
        from trn_agent_boot.trn_boot import _ntff_profile_via_ctypes

        hooks = types.ModuleType("antenv.axon_hooks")
        _state = {}

        def set_axon_ntff_profile_hook(h):
            _state["h"] = h

        def get_axon_ntff_profile_hook():
            return _state.get("h")

        hooks.set_axon_ntff_profile_hook = set_axon_ntff_profile_hook
        hooks.get_axon_ntff_profile_hook = get_axon_ntff_profile_hook
        sys.modules["antenv.axon_hooks"] = hooks
        antenv.axon_hooks = hooks
        hook = _ntff_profile_via_ctypes("/opt/axon/libaxon_pjrt.so")
        if hook is not None:
            set_axon_ntff_profile_hook(hook)
    except Exception:
        pass


def kernel(**inputs):
    global LAST_RESULTS
    from concourse.bass_utils import run_bass_kernel_spmd

    nc = _get_nc()
    in_maps = _stage_inputs(
        inputs["position_ids"], inputs["hidden_states"], inputs["w_qkv"], inputs["w_o"]
    )
    trace = os.environ.get("KERNEL_TRACE", "0") == "1"
    if trace:
        _ensure_ntff_hook()
    res = run_bass_kernel_spmd(
        nc, in_maps, core_ids=list(range(NCORES)), trace=trace
    )
    LAST_RESULTS = res
    outs = [np.asarray(res.results[i]["out"], dtype=np.float32) for i in range(NCORES)]
    full = np.concatenate(outs, axis=0)                             # [S, H]
    return full.reshape(1, S, H)


# revision 4
# speedup vs baseline: 1.0424x; 1.0424x over previous
"""Distributed GQA attention kernel for 8 TRN2 NeuronCores — v2.

Strategy (tensor-parallel over heads, A2A re-shard before o_proj), same
algorithm as v1 but restructured for one continuous PE stream:

  - QKV projection streams k-tiles as the hT DMA lands (kt-outer,
    n-inner over 7 PSUM banks: kv + q-pair-0 chunks 0-2), so the PE
    starts ~6us in instead of waiting for the full 8MB hT.
  - RoPE (neox) reads the qkv PSUM banks directly (no staging copies);
    the "swap" is folded into a host-permuted sin table so rope is
    2 muls + 4 block adds per chunk, mostly bf16 2x DVE ops.
  - Causal mask applied by PRE-INITIALIZING the diagonal 128-col PSUM
    stripe with -400 via a tiny triangle matmul, then accumulating the
    scores on top (start=False).  Kills the per-block mask multiply on
    Vector entirely.  Exp is trimmed to skip fully-masked columns, the
    skipped e columns are zeroed by gpsimd memsets.
  - Attention is software-pipelined (scores kb, PV kb-1) and the PE
    gaps left by the Exp-paced chain are filled with interleaved
    "filler" matmuls: remaining q projections (pair-0 n3 + all of
    pair-1), v transposes, and later the even-kt half of o_proj.
  - Two AllToAlls (one per head pair); pair-0's A2A overlaps pair-1's
    attention; o_proj even-kt chunks run during pair-1 + A2A-1, odd-kt
    chunks after A2A-1.  o_proj partials staged in SBUF f32 so PSUM
    pressure stays at 2 banks.
  - DMA issue spread across the SP/ACT/DVE/Pool sequencers (descriptor
    generation is ~0.6-1us per DMA on one sequencer).
  - Output written bf16, host casts to f32.
"""

import os
import numpy as np
import ml_dtypes

import concourse.bass as bass
import concourse.mybir as mybir
from concourse import bacc, tile
import bass_rust as _br

F32 = mybir.dt.float32
BF16 = mybir.dt.bfloat16
BF16_NP = ml_dtypes.bfloat16

# Problem constants (hardcoded per harness contract)
H = 2048
S = 2048
NH = 32
NKV = 8
HD = 64
Q_SIZE = NH * HD      # 2048
KV_SIZE = NKV * HD    # 512
NCORES = 8
QH = NH // NCORES     # 4 q heads per core
ROPE_THETA = 10000.0

P = 128
KT = H // P           # 16 contraction tiles over H
NQ = 512              # query chunk (matmul moving free dim)
NJC = S // NQ         # 4 query chunks
NKB = S // P          # 16 key tiles
SROWS = S // NCORES   # 256 seq rows per core after A2A
MASKVAL = -400.0      # pre-exp mask bias; *0.125 = -50 under exp
USE_MASKMUL = False   # bisect flag: baseline post-exp mask multiply

_NC_CACHE = None
LAST_RESULTS = None


def _build_nc():
    nc = bacc.Bacc(
        "TRN2",
        target_bir_lowering=False,
        debug=False,
        num_devices=NCORES,
    )

    # ---- I/O ----
    hT_d = nc.dram_tensor("hT", [P, KT * S], BF16, kind="ExternalInput")
    wq_d = nc.dram_tensor("wqkvT", [P, KT * 384], BF16, kind="ExternalInput")
    wo_d = nc.dram_tensor("woT", [P, KT * H], BF16, kind="ExternalInput")
    ropec_d = nc.dram_tensor("ropec", [P, S], BF16, kind="ExternalInput")
    ropes_d = nc.dram_tensor("ropesP", [P, S], BF16, kind="ExternalInput")
    identf_d = nc.dram_tensor("identf", [P, P], F32, kind="ExternalInput")
    tri_d = nc.dram_tensor("tri", [P, P], BF16, kind="ExternalInput")
    identb_d = nc.dram_tensor("identb", [P, P], BF16, kind="ExternalInput")
    masks_d = nc.dram_tensor("masks", [P, 4 * 2 * NQ], BF16, kind="ExternalInput")
    out_d = nc.dram_tensor("out", [SROWS, H], F32, kind="ExternalOutput")

    rg = [list(range(NCORES))]

    with tile.TileContext(nc) as tc:
        with (
            tc.tile_pool(name="dram", bufs=1, space="DRAM") as dram,
            tc.tile_pool(name="const", bufs=1) as const,
            tc.tile_pool(name="qk", bufs=1) as qkpool,
            tc.tile_pool(name="esb", bufs=6) as esb,
            tc.tile_pool(name="small", bufs=3) as small,
            tc.tile_pool(name="outp", bufs=2) as outp,
            tc.tile_pool(name="ropet", bufs=2) as ropet,
        ):
            cc_in = [
                dram.tile([NCORES * P, SROWS], BF16, tag=f"cc_in{p}", name=f"cc_in{p}")
                for p in range(2)
            ]
            cc_out = [
                dram.tile([NCORES * P, SROWS], BF16, tag=f"cc_out{p}", name=f"cc_out{p}")
                for p in range(2)
            ]

            # tiny warm-up collective absorbs the ncfw first-collective delay
            ccw_in = dram.tile([NCORES, 16], BF16, tag="ccw_in")
            ccw_out = dram.tile([NCORES, 16], BF16, tag="ccw_out")
            warm_sb = const.tile([NCORES, 16], BF16, tag="warm_sb")
            nc.vector.memset(warm_sb[:], 0.0)
            nc.sync.dma_start(ccw_in[:], warm_sb[:])
            nc.gpsimd.collective_compute(
                "AllToAll",
                mybir.AluOpType.bypass,
                replica_groups=rg,
                ins=[ccw_in.opt()],
                outs=[ccw_out.opt()],
            )

            # ---- persistent SBUF ----
            qpair = [
                qkpool.tile([P, S], BF16, tag=f"qpair{m}", name=f"qpair{m}")
                for m in range(2)
            ]
            kdup = qkpool.tile([P, S], BF16, tag="kdup")
            kvb = qkpool.tile([P, S], BF16, tag="kvb")
            v_aug = qkpool.tile([P, NKB * 128], BF16, tag="v_aug")
            cc_sb = qkpool.tile([P, KT * SROWS], BF16, tag="cc_sb")
            o_even = qkpool.tile([P, 8 * NQ], F32, tag="o_even")

            # ---- DMA issue: weights+hidden first, spread across sequencers ----
            hq_ctx = tc.tile_pool(name="hq", bufs=1)
            hq = hq_ctx.__enter__()
            wq_sb = hq.tile([P, KT * 384], BF16, tag="wq_sb")
            h_tiles = []
            for kt in range(KT):
                nc.sync.dma_start(
                    wq_sb[:, kt * 384 : (kt + 1) * 384],
                    wq_d[:, kt * 384 : (kt + 1) * 384],
                )
                ht = hq.tile([P, S], BF16, tag=f"h{kt}", name=f"h{kt}")
                # two half-tile sub-DMAs on different sequencers so tiles
                # land incrementally across queues
                nc.sync.dma_start(
                    ht[:, 0 : S // 2], hT_d[:, kt * S : kt * S + S // 2]
                )
                nc.sync.dma_start(
                    ht[:, S // 2 : S], hT_d[:, kt * S + S // 2 : (kt + 1) * S]
                )
                h_tiles.append(ht)
            ropec = const.tile([P, S], BF16, tag="ropec")
            nc.sync.dma_start(ropec[:], ropec_d[:])
            ropesP = const.tile([P, S], BF16, tag="ropesP")
            nc.sync.dma_start(ropesP[:], ropes_d[:])
            identf = const.tile([P, P], F32, tag="identf")
            nc.sync.dma_start(identf[:], identf_d[:])
            identb = const.tile([P, P], BF16, tag="identb")
            nc.sync.dma_start(identb[:], identb_d[:])
            tri = const.tile([P, P], BF16, tag="tri")
            nc.sync.dma_start(tri[:], tri_d[:])
            masks = None
            if USE_MASKMUL:
                masks = const.tile([P, 4 * 2 * NQ], BF16, tag="masks")
                nc.sync.dma_start(masks[:], masks_d[:])

            # fill v_aug with ones once; the v DMA-transposes then overwrite
            # the 64-wide value blocks, leaving each 65th column at 1.0
            nc.gpsimd.memset(v_aug[:], 1.0)

            # ================= Phase 1: streaming QKV =================
            # kv group (m=2) all 4 chunks + q-pair-0 (m=0) chunks 0..2,
            # kt-outer so the PE consumes h tiles as they arrive.
            def rope_chunk(dst, dst_rows, src, n, src_is_chunk=False):
                """RoPE from an SBUF staging copy into dst[:, n*NQ:(n+1)*NQ].
                src is a full-width staging tile (k/v) unless src_is_chunk."""
                c0, c1 = NQ * n, NQ * (n + 1)
                s = src[:, 0:NQ] if src_is_chunk else src[:, c0:c1]
                pc = ropet.tile([P, NQ], BF16, tag="pc", name="pc")
                nc.vector.tensor_mul(
                    pc[0:dst_rows, :], s[0:dst_rows, :], ropec[0:dst_rows, c0:c1]
                )
                swp = ropet.tile([P, NQ], BF16, tag="swp", name="swp")
                for b in range(dst_rows // 64):
                    o = 64 * b
                    nc.vector.tensor_copy(swp[o : o + 32, :], s[o + 32 : o + 64, :])
                    nc.vector.tensor_copy(swp[o + 32 : o + 64, :], s[o : o + 32, :])
                psn = ropet.tile([P, NQ], BF16, tag="psn", name="psn")
                nc.vector.tensor_mul(
                    psn[0:dst_rows, :], swp[0:dst_rows, :], ropesP[0:dst_rows, c0:c1]
                )
                nc.vector.tensor_add(
                    dst[0:dst_rows, c0:c1], pc[0:dst_rows, :], psn[0:dst_rows, :]
                )
                return pc, psn

            ph1_ctx = tc.tile_pool(name="ps1", bufs=1, space="PSUM")
            ps1 = ph1_ctx.__enter__()
            kv_ps = [ps1.tile([P, NQ], F32, tag=f"kv{n}", name=f"kv{n}") for n in range(4)]
            m0_ps = [ps1.tile([P, NQ], F32, tag=f"m0{n}", name=f"m0{n}") for n in range(3)]
            for kt in range(KT):
                for n in range(4):
                    nc.tensor.matmul(
                        kv_ps[n][:],
                        wq_sb[:, kt * 384 + 256 : kt * 384 + 384],
                        h_tiles[kt][:, NQ * n : NQ * (n + 1)],
                        start=(kt == 0),
                        stop=(kt == KT - 1),
                    )
                for n in range(3):
                    nc.tensor.matmul(
                        m0_ps[n][:],
                        wq_sb[:, kt * 384 : kt * 384 + 128],
                        h_tiles[kt][:, NQ * n : NQ * (n + 1)],
                        start=(kt == 0),
                        stop=(kt == KT - 1),
                    )

            # kv/q consumption, deadline order: chunk n gates attention jc=n.
            # ACT: v copies (it is idle until the first exp); DVE: k/q rope
            # direct from PSUM; k row duplication on ACT; v transposed into
            # v_aug by DMA-transpose (XBAR) - no PE or PSUM involved.
            q0stg = [
                qkpool.tile([P, NQ], BF16, tag=f"q0stg{n}", name=f"q0stg{n}")
                for n in range(3)
            ]
            # stage all 7 banks to SBUF up front so the attention pools can
            # allocate PSUM immediately; ropes then run from SBUF, overlapped
            # with attention
            for n in range(4):
                nc.scalar.copy(
                    kvb[:, NQ * n : NQ * (n + 1)], kv_ps[n][:]
                )
                if n < 3:
                    nc.scalar.copy(q0stg[n][:], m0_ps[n][:])
            for n in range(4):
                pc, psn = rope_chunk(kdup, 64, kvb, n)
                nc.vector.tensor_add(
                    kdup[64:128, NQ * n : NQ * (n + 1)], pc[0:64, :], psn[0:64, :]
                )
                if n < 3:
                    rope_chunk(qpair[0], 128, q0stg[n], n, src_is_chunk=True)
            # jc0's v transposes (kb 0-3) in the spare phase-1 bank
            vtps = ps1.tile([P, NKB * 64], BF16, tag="vtps", name="vtps")
            for kb in range(NKB):
                nc.tensor.transpose(
                    vtps[:, 64 * kb : 64 * (kb + 1)],
                    kvb[64:128, P * kb : P * (kb + 1)],
                    identb[64:128, 64:128],
                )
            nc.vector.tensor_copy(
                v_aug[:].rearrange("p (kb c) -> p kb c", kb=NKB)[:, :, 0:64],
                vtps[:].rearrange("p (kb c) -> p kb c", kb=NKB),
            )
            # cols 64-127 of each block stay 1.0: the PV matmul replicates the
            # softmax denominator into PSUM rows 64-127 (no broadcast needed)
            ph1_ctx.__exit__(None, None, None)

            # w_o streamed at the pair-0/pair-1 boundary (SBUF freed by h)
            wo_ctx = None
            wo_tiles = []

            def emit_wo_stream():
                nonlocal wo_ctx
                wo_ctx = tc.tile_pool(name="wo_stream", bufs=4)
                wo_stream = wo_ctx.__enter__()
                for n in range(NJC):
                    wos = wo_stream.tile(
                        [P, KT * NQ], BF16, tag="wos", name=f"wos{n}"
                    )
                    for kg in range(4):
                        nc.sync.dma_start(
                            wos[:].rearrange("p (kt c) -> p kt c", kt=KT)[
                                :, 4 * kg : 4 * (kg + 1), :
                            ],
                            wo_d[:].rearrange("p (kt c) -> p kt c", kt=KT)[
                                :, 4 * kg : 4 * (kg + 1), NQ * n : NQ * (n + 1)
                            ],
                        )
                    wo_tiles.append(wos)

            # ============ Phase 2: attention with filler interleave ============
            ps_s_ctx = tc.tile_pool(name="ps_s", bufs=2, space="PSUM")
            ps_s = ps_s_ctx.__enter__()
            ps_ctx_ctx = tc.tile_pool(name="ps_ctx", bufs=1, space="PSUM")
            ps_ctx = ps_ctx_ctx.__enter__()
            fil_ctx = tc.tile_pool(name="fil", bufs=2, space="PSUM")
            fil = fil_ctx.__enter__()

            # ---- filler op queue: list of closures, each emits 1 PE op ----
            filler_q = []
            fil_state = {"tile": None, "count": 0, "kind": None, "n": None}

            def emit_qproj_mm(dst_pair, m_off, n, kt, fil_tile):
                nc.tensor.matmul(
                    fil_tile[:],
                    wq_sb[:, kt * 384 + m_off : kt * 384 + m_off + 128],
                    h_tiles[kt][:, NQ * n : NQ * (n + 1)],
                    start=(kt == 0),
                    stop=(kt == KT - 1),
                )

            def make_qproj_chunk(dst_pair, m_off, n):
                # 16 matmuls accumulating into one fil tile, then rope
                ops = []
                holder = {}

                def first(kt=0):
                    holder["t"] = fil.tile([P, NQ], F32, tag="fil", name=f"qp{dst_pair}{n}")
                    emit_qproj_mm(dst_pair, m_off, n, 0, holder["t"])
                ops.append(first)
                for kt in range(1, KT):
                    ops.append(lambda kt=kt: emit_qproj_mm(
                        dst_pair, m_off, n, kt, holder["t"]))

                def last():
                    stg = ropet.tile([P, NQ], BF16, tag="stg", name="stg")
                    nc.vector.tensor_copy(stg[:], holder["t"][:])
                    rope_chunk(qpair[dst_pair], 128, stg, n, src_is_chunk=True)
                ops.append(last)   # DVE ops ride along with the last PE op
                return ops

            def make_oproj_chunk(c, parity):
                # 8 matmuls (kts of one parity) for output chunk c=(n,m)
                n, m = c // 2, c % 2
                ops = []
                holder = {}

                def mm(kk, first):
                    if first:
                        holder["t"] = fil.tile([P, NQ], F32, tag="fil", name=f"op{parity}{c}")
                    kt = 2 * kk + parity
                    nc.tensor.matmul(
                        holder["t"][:],
                        cc_sb[:, kt * SROWS + P * m : kt * SROWS + P * (m + 1)],
                        wo_tiles[n][:, kt * NQ : (kt + 1) * NQ],
                        start=(kk == 0),
                        stop=(kk == 7),
                    )
                ops.append(lambda: mm(0, True))
                for kk in range(1, 8):
                    ops.append(lambda kk=kk: mm(kk, False))

                if parity == 0:
                    def last():
                        nc.vector.tensor_copy(
                            o_even[:, NQ * c : NQ * (c + 1)], holder["t"][:]
                        )
                    ops.append(last)
                else:
                    def last():
                        ot = outp.tile([P, NQ], F32, tag="ot")
                        nc.vector.tensor_add(
                            ot[:], o_even[:, NQ * c : NQ * (c + 1)], holder["t"][:]
                        )
                        for oh in range(2):
                            nc.sync.dma_start(
                                out_d[
                                    P * m : P * (m + 1),
                                    NQ * n + 256 * oh : NQ * n + 256 * (oh + 1),
                                ],
                                ot[:, 256 * oh : 256 * (oh + 1)],
                            )
                    ops.append(last)
                return ops

            # pair-0 fillers, deadline order: v transposes for jc1-3, then
            # q-proj pair0-n3, then all of pair-1's q projections
            filler_q.append(make_qproj_chunk(0, 0, 3))
            for n in range(4):
                filler_q.append(make_qproj_chunk(1, 128, n))
            # flatten with chunk boundaries preserved (chunks already ordered
            # by deadline; ops within a chunk must stay in order)
            flat_fillers = [op for ch in filler_q for op in ch]
            fill_pos = {"i": 0}

            def emit_fillers(k):
                for _ in range(k):
                    if fill_pos["i"] >= len(flat_fillers):
                        return
                    flat_fillers[fill_pos["i"]]()
                    fill_pos["i"] += 1

            last_scatter = [None, None]
            early_p1_scatter = [None]
            cc_insts = []
            for p in range(2):
                if p == 1:
                    # drain leftover q-proj fillers (they read h), free the
                    # h/wq SBUF, then start streaming w_o into that space
                    emit_fillers(len(flat_fillers) - fill_pos["i"])
                    hq_ctx.__exit__(None, None, None)
                    emit_wo_stream()
                for jc in range(NJC):
                    nkb = 4 * (jc + 1)
                    ctxs = [
                        ps_ctx.tile([P, NQ], F32, tag=f"ctx{hh}", name=f"ctx{hh}")
                        for hh in range(2)
                    ]
                    sp_prev = None
                    e_prev = None
                    for kb in range(nkb):
                        d = kb - 4 * jc
                        emit_fillers(2 if p == 0 else 3)
                        sp = ps_s.tile([P, 2 * NQ], F32, tag="sp", name="sp")
                        if d >= 0 and USE_MASKMUL:
                            for hh in range(2):
                                base = 64 * hh
                                nc.tensor.matmul(
                                    sp[:, NQ * hh : NQ * (hh + 1)],
                                    kdup[base : base + 64, P * kb : P * (kb + 1)],
                                    qpair[p][base : base + 64, NQ * jc : NQ * (jc + 1)],
                                    start=True,
                                    stop=True,
                                )
                        elif d >= 0:
                            for hh in range(2):
                                co = NQ * hh + 128 * d
                                nc.tensor.matmul(
                                    sp[:, co : co + 128],
                                    identb[:],
                                    tri[:],
                                    start=True,
                                    stop=False,
                                    skip_group_check=True,
                                )
                            for hh in range(2):
                                base = 64 * hh
                                co = NQ * hh + 128 * d
                                nc.tensor.matmul(
                                    sp[:, co : co + 128],
                                    kdup[base : base + 64, P * kb : P * (kb + 1)],
                                    qpair[p][
                                        base : base + 64,
                                        NQ * jc + 128 * d : NQ * jc + 128 * (d + 1),
                                    ],
                                    start=False,
                                    stop=True,
                                    skip_group_check=True,
                                )
                            if d < 3:
                                for hh in range(2):
                                    base = 64 * hh
                                    co = NQ * hh + 128 * (d + 1)
                                    nc.tensor.matmul(
                                        sp[:, co : NQ * (hh + 1)],
                                        kdup[base : base + 64, P * kb : P * (kb + 1)],
                                        qpair[p][
                                            base : base + 64,
                                            NQ * jc + 128 * (d + 1) : NQ * (jc + 1),
                                        ],
                                        start=True,
                                        stop=True,
                                        skip_group_check=True,
                                    )
                        else:
                            for hh in range(2):
                                base = 64 * hh
                                nc.tensor.matmul(
                                    sp[:, NQ * hh : NQ * (hh + 1)],
                                    kdup[base : base + 64, P * kb : P * (kb + 1)],
                                    qpair[p][base : base + 64, NQ * jc : NQ * (jc + 1)],
                                    start=True,
                                    stop=True,
                                )
                        # PV for previous kb (software pipeline depth 1);
                        # diagonal blocks contribute nothing to the columns
                        # left of their stripe, so trim both e and ctx there
                        if kb > 0:
                            off = 128 * (d - 1) if (d >= 1 and not USE_MASKMUL) else 0
                            for hh in range(2):
                                nc.tensor.matmul(
                                    ctxs[hh][:, off:NQ],
                                    v_aug[:, (kb - 1) * 128 : kb * 128],
                                    e_prev[:, NQ * hh + off : NQ * (hh + 1)],
                                    start=(kb - 1 == 0),
                                    stop=False,
                                    skip_group_check=True,
                                )
                        # exp, trimmed to the live columns (per-hh 2D APs)
                        e = esb.tile([P, 2 * NQ], BF16, tag="e", name="e")
                        if USE_MASKMUL:
                            nc.scalar.activation(
                                e[:], sp[:], mybir.ActivationFunctionType.Exp,
                                scale=0.125,
                            )
                            if d >= 0:
                                nc.vector.tensor_mul(
                                    e[:], e[:],
                                    masks[:, 2 * NQ * d : 2 * NQ * (d + 1)],
                                )
                        elif d >= 1:
                            for hh in range(2):
                                nc.scalar.activation(
                                    e[:, NQ * hh + 128 * d : NQ * (hh + 1)],
                                    sp[:, NQ * hh + 128 * d : NQ * (hh + 1)],
                                    mybir.ActivationFunctionType.Exp,
                                    scale=0.125,
                                )
                        elif not USE_MASKMUL:
                            nc.scalar.activation(
                                e[:], sp[:], mybir.ActivationFunctionType.Exp,
                                scale=0.125,
                            )
                        sp_prev, e_prev = sp, e
                    # final PV (always a d=3 diagonal block)
                    foff = 0 if USE_MASKMUL else 384
                    for hh in range(2):
                        nc.tensor.matmul(
                            ctxs[hh][:, foff:NQ],
                            v_aug[:, (nkb - 1) * 128 : nkb * 128],
                            e_prev[:, NQ * hh + foff : NQ * (hh + 1)],
                            start=False,
                            stop=True,
                            skip_group_check=True,
                        )
                    # epilogue: normalize + scatter
                    for hh in range(2):
                        cb = small.tile([P, NQ], F32, tag="cb")
                        nc.vector.tensor_copy(cb[:], ctxs[hh][:, :])
                        # denominators sit replicated in rows 64-127; stage at
                        # base 0 (custom DVE ops misread at nonzero base)
                        rb = small.tile([64, NQ], F32, tag="rb")
                        nc.vector.tensor_copy(rb[:], cb[64:128, :])
                        rec = small.tile([64, NQ], F32, tag="rec")
                        nc.vector.reciprocal_approx_fast(rec[:], rb[:])
                        ctxn = small.tile([64, NQ], BF16, tag="ctxn")
                        nc.vector.tensor_mul(ctxn[:], cb[0:64, :], rec[:])
                        for half in range(2):
                            j = 2 * jc + half
                            last_scatter[p] = nc.sync.dma_start(
                                cc_in[p][
                                    P * j + 64 * hh : P * j + 64 * (hh + 1), :
                                ],
                                ctxn[:, SROWS * half : SROWS * (half + 1)],
                            )
                            if p == 1 and jc == 0:
                                early_p1_scatter[0] = last_scatter[p]
                # A2A for this pair
                cc_insts.append(
                    nc.gpsimd.collective_compute(
                        "AllToAll",
                        mybir.AluOpType.bypass,
                        replica_groups=rg,
                        ins=[cc_in[p].opt()],
                        outs=[cc_out[p].opt()],
                    )
                )

            # ---- tail (baseline structure): drain fillers, then cc_sb loads
            # for both parities; each waits on its own A2A, so even-kt o_proj
            # chunks execute during A2A-1 ----
            emit_fillers(len(flat_fillers) - fill_pos["i"])
            # even loads first: odd loads wait on A2A-1 and would otherwise
            # head-of-line block the even data the A2A-1-covering even o_proj
            # pass needs
            for par in range(2):
                for j in range(NCORES):
                    dma = nc.sync.dma_start(
                        cc_sb[
                            :,
                            (2 * j + par) * SROWS : (2 * j + par) * SROWS + SROWS,
                        ],
                        cc_out[par][j * P : (j + 1) * P, :],
                    )
                    anchor = early_p1_scatter[0] if par == 0 else last_scatter[1]
                    _br.add_dep_helper(
                        dma.ins, anchor.ins, sync=True,
                        reason="cc_sb read ordered behind scatter traffic",
                    )
            for c in range(8):
                for op in make_oproj_chunk(c, 0):
                    op()
            # odd passes + combine + output
            for c in range(8):
                for op in make_oproj_chunk(c, 1):
                    op()

            fil_ctx.__exit__(None, None, None)
            ps_ctx_ctx.__exit__(None, None, None)
            ps_s_ctx.__exit__(None, None, None)
            wo_ctx.__exit__(None, None, None)

    nc.compile()
    return nc


def _get_nc():
    global _NC_CACHE
    if _NC_CACHE is None:
        _NC_CACHE = _build_nc()
    return _NC_CACHE


def _stage_inputs(position_ids, hidden_states, w_qkv, w_o):
    """Host-side sharding / layout staging. Returns in_maps for 8 cores."""
    pos = np.asarray(position_ids)[0].astype(np.float32)            # [S]
    hidden = np.asarray(hidden_states, dtype=np.float32)[0]         # [S, H]
    w_qkv = np.asarray(w_qkv, dtype=np.float32)                     # [3072, H]
    w_o = np.asarray(w_o, dtype=np.float32)                         # [H, Q_SIZE]

    # hT tiles: [H, S] -> [128, KT*S] (k-tile kt at cols kt*S..)
    hT = np.ascontiguousarray(hidden.T)
    hT_r = np.ascontiguousarray(
        hT.reshape(KT, P, S).transpose(1, 0, 2).reshape(P, KT * S)
    ).astype(BF16_NP)

    # w_o.T tiles: [Q_SIZE, H] -> [128, KT*H]
    woT = np.ascontiguousarray(w_o.T)
    woT_r = np.ascontiguousarray(
        woT.reshape(KT, P, H).transpose(1, 0, 2).reshape(P, KT * H)
    ).astype(BF16_NP)

    # rope tables in [d, s] layout for a [128 = 2 heads x 64] tile; the sin
    # product reads its x input at the partner rows, so the table rows hold
    # the sign for the OUTPUT row: [-sin; +sin] per 64-row head block
    inv_freq = (1.0 / (ROPE_THETA ** (np.arange(0, HD, 2, dtype=np.float32) / HD)))
    ang = pos[:, None] * inv_freq[None, :]                          # [S, 32]
    cosT = np.cos(ang).T.astype(np.float32)                         # [32, S]
    sinT = np.sin(ang).T.astype(np.float32)
    ropec = np.concatenate([cosT, cosT, cosT, cosT], axis=0)        # [128, S]
    ropesP = np.concatenate([-sinT, sinT, -sinT, sinT], axis=0)     # [128, S]

    ident = np.eye(P, dtype=np.float32)
    f = np.arange(NQ)
    mask_list = []
    for d in range(4):
        pp = np.arange(P)[:, None] + 128 * d
        mk = (pp <= f[None, :]).astype(BF16_NP)
        mask_list.append(mk)
        mask_list.append(mk)
    masks_np = np.concatenate(mask_list, axis=1)                    # [128, 8*NQ]
    tri = np.where(
        np.arange(P)[:, None] > np.arange(P)[None, :], MASKVAL, 0.0
    ).astype(np.float32)

    in_maps = []
    for i in range(NCORES):
        rows_q = w_qkv[QH * HD * i : QH * HD * (i + 1)]             # [256, H]
        row_k = w_qkv[Q_SIZE + HD * i : Q_SIZE + HD * (i + 1)]      # [64, H]
        row_v = w_qkv[Q_SIZE + KV_SIZE + HD * i : Q_SIZE + KV_SIZE + HD * (i + 1)]
        wshard = np.concatenate([rows_q, row_k, row_v], axis=0)     # [384, H]
        wqkvT = np.ascontiguousarray(wshard.T)                      # [H, 384]
        wqkvT_r = np.ascontiguousarray(
            wqkvT.reshape(KT, P, 384).transpose(1, 0, 2).reshape(P, KT * 384)
        ).astype(BF16_NP)
        in_maps.append(
            {
                "hT": hT_r,
                "wqkvT": wqkvT_r,
                "woT": woT_r,
                "ropec": ropec.astype(BF16_NP),
                "ropesP": ropesP.astype(BF16_NP),
                "identf": ident,
                "identb": ident.astype(BF16_NP),
                "tri": tri.astype(BF16_NP),
                "masks": masks_np,
            }
        )
    return in_maps


def _ensure_ntff_hook():
    """The container's antenv stub lacks axon_hooks, so trn_boot silently
    skipped NTFF hook registration. Recreate the module and register the
    ctypes-based hook so run_bass_kernel_spmd(trace=True) can profile."""
    import sys
    import types

    if "antenv.axon_hooks" in sys.modules:
        return
    try:
        import antenv
        from trn_agent_boot.trn_boot import _ntff_profile_via_ctypes

        hooks = types.ModuleType("antenv.axon_hooks")
        _state = {}

        def set_axon_ntff_profile_hook(h):
            _state["h"] = h

        def get_axon_ntff_profile_hook():
            return _state.get("h")

        hooks.set_axon_ntff_profile_hook = set_axon_ntff_profile_hook
        hooks.get_axon_ntff_profile_hook = get_axon_ntff_profile_hook
        sys.modules["antenv.axon_hooks"] = hooks
        antenv.axon_hooks = hooks
        hook = _ntff_profile_via_ctypes("/opt/axon/libaxon_pjrt.so")
        if hook is not None:
            set_axon_ntff_profile_hook(hook)
    except Exception:
        pass


def kernel(**inputs):
    global LAST_RESULTS
    from concourse.bass_utils import run_bass_kernel_spmd

    nc = _get_nc()
    in_maps = _stage_inputs(
        inputs["position_ids"], inputs["hidden_states"], inputs["w_qkv"], inputs["w_o"]
    )
    trace = os.environ.get("KERNEL_TRACE", "0") == "1"
    if trace:
        _ensure_ntff_hook()
    res = run_bass_kernel_spmd(
        nc, in_maps, core_ids=list(range(NCORES)), trace=trace
    )
    LAST_RESULTS = res
    outs = [np.asarray(res.results[i]["out"], dtype=np.float32) for i in range(NCORES)]
    full = np.concatenate(outs, axis=0)                             # [S, H]
    return full.reshape(1, S, H)


# revision 5
# speedup vs baseline: 1.0497x; 1.0070x over previous
"""Distributed GQA attention kernel for 8 TRN2 NeuronCores — v2.

Strategy (tensor-parallel over heads, A2A re-shard before o_proj), same
algorithm as v1 but restructured for one continuous PE stream:

  - QKV projection streams k-tiles as the hT DMA lands (kt-outer,
    n-inner over 7 PSUM banks: kv + q-pair-0 chunks 0-2), so the PE
    starts ~6us in instead of waiting for the full 8MB hT.
  - RoPE (neox) reads the qkv PSUM banks directly (no staging copies);
    the "swap" is folded into a host-permuted sin table so rope is
    2 muls + 4 block adds per chunk, mostly bf16 2x DVE ops.
  - Causal mask applied by PRE-INITIALIZING the diagonal 128-col PSUM
    stripe with -400 via a tiny triangle matmul, then accumulating the
    scores on top (start=False).  Kills the per-block mask multiply on
    Vector entirely.  Exp is trimmed to skip fully-masked columns, the
    skipped e columns are zeroed by gpsimd memsets.
  - Attention is software-pipelined (scores kb, PV kb-1) and the PE
    gaps left by the Exp-paced chain are filled with interleaved
    "filler" matmuls: remaining q projections (pair-0 n3 + all of
    pair-1), v transposes, and later the even-kt half of o_proj.
  - Two AllToAlls (one per head pair); pair-0's A2A overlaps pair-1's
    attention; o_proj even-kt chunks run during pair-1 + A2A-1, odd-kt
    chunks after A2A-1.  o_proj partials staged in SBUF f32 so PSUM
    pressure stays at 2 banks.
  - DMA issue spread across the SP/ACT/DVE/Pool sequencers (descriptor
    generation is ~0.6-1us per DMA on one sequencer).
  - Output written bf16, host casts to f32.
"""

import os
import numpy as np
import ml_dtypes

import concourse.bass as bass
import concourse.mybir as mybir
from concourse import bacc, tile
import bass_rust as _br

F32 = mybir.dt.float32
BF16 = mybir.dt.bfloat16
BF16_NP = ml_dtypes.bfloat16

# Problem constants (hardcoded per harness contract)
H = 2048
S = 2048
NH = 32
NKV = 8
HD = 64
Q_SIZE = NH * HD      # 2048
KV_SIZE = NKV * HD    # 512
NCORES = 8
QH = NH // NCORES     # 4 q heads per core
ROPE_THETA = 10000.0

P = 128
KT = H // P           # 16 contraction tiles over H
NQ = 512              # query chunk (matmul moving free dim)
NJC = S // NQ         # 4 query chunks
NKB = S // P          # 16 key tiles
SROWS = S // NCORES   # 256 seq rows per core after A2A
MASKVAL = -400.0      # pre-exp mask bias; *0.125 = -50 under exp
USE_MASKMUL = False   # bisect flag: baseline post-exp mask multiply

_NC_CACHE = None
LAST_RESULTS = None


def _build_nc():
    nc = bacc.Bacc(
        "TRN2",
        target_bir_lowering=False,
        debug=False,
        num_devices=NCORES,
    )

    # ---- I/O ----
    hT_d = nc.dram_tensor("hT", [P, KT * S], BF16, kind="ExternalInput")
    wq_d = nc.dram_tensor("wqkvT", [P, KT * 384], BF16, kind="ExternalInput")
    wo_d = nc.dram_tensor("woT", [P, KT * H], BF16, kind="ExternalInput")
    ropec_d = nc.dram_tensor("ropec", [P, S], BF16, kind="ExternalInput")
    ropes_d = nc.dram_tensor("ropesP", [P, S], BF16, kind="ExternalInput")
    identf_d = nc.dram_tensor("identf", [P, P], F32, kind="ExternalInput")
    tri_d = nc.dram_tensor("tri", [P, P], BF16, kind="ExternalInput")
    identb_d = nc.dram_tensor("identb", [P, P], BF16, kind="ExternalInput")
    masks_d = nc.dram_tensor("masks", [P, 4 * 2 * NQ], BF16, kind="ExternalInput")
    out_d = nc.dram_tensor("out", [SROWS, H], F32, kind="ExternalOutput")

    rg = [list(range(NCORES))]

    with tile.TileContext(nc) as tc:
        with (
            tc.tile_pool(name="dram", bufs=1, space="DRAM") as dram,
            tc.tile_pool(name="const", bufs=1) as const,
            tc.tile_pool(name="qk", bufs=1) as qkpool,
            tc.tile_pool(name="esb", bufs=6) as esb,
            tc.tile_pool(name="small", bufs=3) as small,
            tc.tile_pool(name="outp", bufs=2) as outp,
            tc.tile_pool(name="ropet", bufs=2) as ropet,
        ):
            cc_in = [
                dram.tile([NCORES * P, SROWS], BF16, tag=f"cc_in{p}", name=f"cc_in{p}")
                for p in range(2)
            ]
            cc_out = [
                dram.tile([NCORES * P, SROWS], BF16, tag=f"cc_out{p}", name=f"cc_out{p}")
                for p in range(2)
            ]

            # tiny warm-up collective absorbs the ncfw first-collective delay
            ccw_in = dram.tile([NCORES, 16], BF16, tag="ccw_in")
            ccw_out = dram.tile([NCORES, 16], BF16, tag="ccw_out")
            warm_sb = const.tile([NCORES, 16], BF16, tag="warm_sb")
            nc.vector.memset(warm_sb[:], 0.0)
            nc.sync.dma_start(ccw_in[:], warm_sb[:])
            nc.gpsimd.collective_compute(
                "AllToAll",
                mybir.AluOpType.bypass,
                replica_groups=rg,
                ins=[ccw_in.opt()],
                outs=[ccw_out.opt()],
            )

            # ---- persistent SBUF ----
            qpair = [
                qkpool.tile([P, S], BF16, tag=f"qpair{m}", name=f"qpair{m}")
                for m in range(2)
            ]
            kdup = qkpool.tile([P, S], BF16, tag="kdup")
            kvb = qkpool.tile([P, S], BF16, tag="kvb")
            v_aug = qkpool.tile([P, NKB * 128], BF16, tag="v_aug")
            cc_sb = qkpool.tile([P, KT * SROWS], BF16, tag="cc_sb")
            o_even = qkpool.tile([P, 8 * NQ], F32, tag="o_even")

            # ---- DMA issue: weights+hidden first, spread across sequencers ----
            hq_ctx = tc.tile_pool(name="hq", bufs=1)
            hq = hq_ctx.__enter__()
            wq_sb = hq.tile([P, KT * 384], BF16, tag="wq_sb")
            h_tiles = []
            for kt in range(KT):
                nc.sync.dma_start(
                    wq_sb[:, kt * 384 : (kt + 1) * 384],
                    wq_d[:, kt * 384 : (kt + 1) * 384],
                )
                ht = hq.tile([P, S], BF16, tag=f"h{kt}", name=f"h{kt}")
                # two half-tile sub-DMAs on different sequencers so tiles
                # land incrementally across queues
                nc.sync.dma_start(
                    ht[:, 0 : S // 2], hT_d[:, kt * S : kt * S + S // 2]
                )
                nc.sync.dma_start(
                    ht[:, S // 2 : S], hT_d[:, kt * S + S // 2 : (kt + 1) * S]
                )
                h_tiles.append(ht)
            ropec = const.tile([P, S], BF16, tag="ropec")
            nc.sync.dma_start(ropec[:], ropec_d[:])
            ropesP = const.tile([P, S], BF16, tag="ropesP")
            nc.sync.dma_start(ropesP[:], ropes_d[:])
            identf = const.tile([P, P], F32, tag="identf")
            nc.sync.dma_start(identf[:], identf_d[:])
            identb = const.tile([P, P], BF16, tag="identb")
            nc.sync.dma_start(identb[:], identb_d[:])
            tri = const.tile([P, P], BF16, tag="tri")
            nc.sync.dma_start(tri[:], tri_d[:])
            masks = None
            if USE_MASKMUL:
                masks = const.tile([P, 4 * 2 * NQ], BF16, tag="masks")
                nc.sync.dma_start(masks[:], masks_d[:])

            # fill v_aug with ones once; the v DMA-transposes then overwrite
            # the 64-wide value blocks, leaving each 65th column at 1.0
            nc.gpsimd.memset(v_aug[:], 1.0)

            # ================= Phase 1: streaming QKV =================
            # kv group (m=2) all 4 chunks + q-pair-0 (m=0) chunks 0..2,
            # kt-outer so the PE consumes h tiles as they arrive.
            def rope_chunk(dst, dst_rows, src, n, src_is_chunk=False):
                """RoPE from an SBUF staging copy into dst[:, n*NQ:(n+1)*NQ].
                src is a full-width staging tile (k/v) unless src_is_chunk."""
                c0, c1 = NQ * n, NQ * (n + 1)
                s = src[:, 0:NQ] if src_is_chunk else src[:, c0:c1]
                pc = ropet.tile([P, NQ], BF16, tag="pc", name="pc")
                nc.vector.tensor_mul(
                    pc[0:dst_rows, :], s[0:dst_rows, :], ropec[0:dst_rows, c0:c1]
                )
                swp = ropet.tile([P, NQ], BF16, tag="swp", name="swp")
                for b in range(dst_rows // 64):
                    o = 64 * b
                    nc.vector.tensor_copy(swp[o : o + 32, :], s[o + 32 : o + 64, :])
                    nc.vector.tensor_copy(swp[o + 32 : o + 64, :], s[o : o + 32, :])
                psn = ropet.tile([P, NQ], BF16, tag="psn", name="psn")
                nc.vector.tensor_mul(
                    psn[0:dst_rows, :], swp[0:dst_rows, :], ropesP[0:dst_rows, c0:c1]
                )
                nc.vector.tensor_add(
                    dst[0:dst_rows, c0:c1], pc[0:dst_rows, :], psn[0:dst_rows, :]
                )
                return pc, psn

            ph1_ctx = tc.tile_pool(name="ps1", bufs=1, space="PSUM")
            ps1 = ph1_ctx.__enter__()
            kv_ps = [ps1.tile([P, NQ], F32, tag=f"kv{n}", name=f"kv{n}") for n in range(4)]
            m0_ps = [ps1.tile([P, NQ], F32, tag=f"m0{n}", name=f"m0{n}") for n in range(3)]
            for kt in range(KT):
                for n in range(4):
                    nc.tensor.matmul(
                        kv_ps[n][:],
                        wq_sb[:, kt * 384 + 256 : kt * 384 + 384],
                        h_tiles[kt][:, NQ * n : NQ * (n + 1)],
                        start=(kt == 0),
                        stop=(kt == KT - 1),
                    )
                for n in range(3):
                    nc.tensor.matmul(
                        m0_ps[n][:],
                        wq_sb[:, kt * 384 : kt * 384 + 128],
                        h_tiles[kt][:, NQ * n : NQ * (n + 1)],
                        start=(kt == 0),
                        stop=(kt == KT - 1),
                    )

            # kv/q consumption, deadline order: chunk n gates attention jc=n.
            # ACT: v copies (it is idle until the first exp); DVE: k/q rope
            # direct from PSUM; k row duplication on ACT; v transposed into
            # v_aug by DMA-transpose (XBAR) - no PE or PSUM involved.
            q0stg = [
                qkpool.tile([P, NQ], BF16, tag=f"q0stg{n}", name=f"q0stg{n}")
                for n in range(3)
            ]
            # stage all 7 banks to SBUF up front so the attention pools can
            # allocate PSUM immediately; ropes then run from SBUF, overlapped
            # with attention
            for n in range(4):
                nc.scalar.copy(
                    kvb[:, NQ * n : NQ * (n + 1)], kv_ps[n][:]
                )
                if n < 3:
                    nc.scalar.copy(q0stg[n][:], m0_ps[n][:])
            for n in range(4):
                pc, psn = rope_chunk(kdup, 64, kvb, n)
                nc.vector.tensor_add(
                    kdup[64:128, NQ * n : NQ * (n + 1)], pc[0:64, :], psn[0:64, :]
                )
                if n < 3:
                    rope_chunk(qpair[0], 128, q0stg[n], n, src_is_chunk=True)
            # jc0's v transposes (kb 0-3) in the spare phase-1 bank
            vtps = ps1.tile([P, NKB * 64], BF16, tag="vtps", name="vtps")
            for kb in range(NKB):
                nc.tensor.transpose(
                    vtps[:, 64 * kb : 64 * (kb + 1)],
                    kvb[64:128, P * kb : P * (kb + 1)],
                    identb[64:128, 64:128],
                )
            nc.vector.tensor_copy(
                v_aug[:].rearrange("p (kb c) -> p kb c", kb=NKB)[:, :, 0:64],
                vtps[:].rearrange("p (kb c) -> p kb c", kb=NKB),
            )
            # cols 64-127 of each block stay 1.0: the PV matmul replicates the
            # softmax denominator into PSUM rows 64-127 (no broadcast needed)
            ph1_ctx.__exit__(None, None, None)

            # w_o streamed at the pair-0/pair-1 boundary (SBUF freed by h)
            wo_ctx = None
            wo_tiles = []

            def emit_wo_stream():
                nonlocal wo_ctx
                wo_ctx = tc.tile_pool(name="wo_stream", bufs=4)
                wo_stream = wo_ctx.__enter__()
                for n in range(NJC):
                    wos = wo_stream.tile(
                        [P, KT * NQ], BF16, tag="wos", name=f"wos{n}"
                    )
                    wo_tiles.append(wos)

            def emit_wo_dmas(n):
                # one tile's sub-DMAs per pair-1 jc: staggers the 8MB wo
                # stream so it doesn't saturate the queues under A2A-0
                wos = wo_tiles[n]
                for kg in range(4):
                    nc.sync.dma_start(
                        wos[:].rearrange("p (kt c) -> p kt c", kt=KT)[
                            :, 4 * kg : 4 * (kg + 1), :
                        ],
                        wo_d[:].rearrange("p (kt c) -> p kt c", kt=KT)[
                            :, 4 * kg : 4 * (kg + 1), NQ * n : NQ * (n + 1)
                        ],
                    )

            # ============ Phase 2: attention with filler interleave ============
            ps_s_ctx = tc.tile_pool(name="ps_s", bufs=2, space="PSUM")
            ps_s = ps_s_ctx.__enter__()
            ps_ctx_ctx = tc.tile_pool(name="ps_ctx", bufs=1, space="PSUM")
            ps_ctx = ps_ctx_ctx.__enter__()
            fil_ctx = tc.tile_pool(name="fil", bufs=2, space="PSUM")
            fil = fil_ctx.__enter__()

            # ---- filler op queue: list of closures, each emits 1 PE op ----
            filler_q = []
            fil_state = {"tile": None, "count": 0, "kind": None, "n": None}

            def emit_qproj_mm(dst_pair, m_off, n, kt, fil_tile):
                nc.tensor.matmul(
                    fil_tile[:],
                    wq_sb[:, kt * 384 + m_off : kt * 384 + m_off + 128],
                    h_tiles[kt][:, NQ * n : NQ * (n + 1)],
                    start=(kt == 0),
                    stop=(kt == KT - 1),
                )

            def make_qproj_chunk(dst_pair, m_off, n):
                # 16 matmuls accumulating into one fil tile, then rope
                ops = []
                holder = {}

                def first(kt=0):
                    holder["t"] = fil.tile([P, NQ], F32, tag="fil", name=f"qp{dst_pair}{n}")
                    emit_qproj_mm(dst_pair, m_off, n, 0, holder["t"])
                ops.append(first)
                for kt in range(1, KT):
                    ops.append(lambda kt=kt: emit_qproj_mm(
                        dst_pair, m_off, n, kt, holder["t"]))

                def last():
                    stg = ropet.tile([P, NQ], BF16, tag="stg", name="stg")
                    nc.vector.tensor_copy(stg[:], holder["t"][:])
                    rope_chunk(qpair[dst_pair], 128, stg, n, src_is_chunk=True)
                ops.append(last)   # DVE ops ride along with the last PE op
                return ops

            def make_oproj_chunk(c, parity):
                # 8 matmuls (kts of one parity) for output chunk c=(n,m)
                n, m = c // 2, c % 2
                ops = []
                holder = {}

                def mm(kk, first):
                    if first:
                        holder["t"] = fil.tile([P, NQ], F32, tag="fil", name=f"op{parity}{c}")
                    kt = 2 * kk + parity
                    nc.tensor.matmul(
                        holder["t"][:],
                        cc_sb[:, kt * SROWS + P * m : kt * SROWS + P * (m + 1)],
                        wo_tiles[n][:, kt * NQ : (kt + 1) * NQ],
                        start=(kk == 0),
                        stop=(kk == 7),
                    )
                ops.append(lambda: mm(0, True))
                for kk in range(1, 8):
                    ops.append(lambda kk=kk: mm(kk, False))

                if parity == 0:
                    def last():
                        nc.vector.tensor_copy(
                            o_even[:, NQ * c : NQ * (c + 1)], holder["t"][:]
                        )
                    ops.append(last)
                else:
                    def last():
                        ot = outp.tile([P, NQ], F32, tag="ot")
                        nc.vector.tensor_add(
                            ot[:], o_even[:, NQ * c : NQ * (c + 1)], holder["t"][:]
                        )
                        for oh in range(2):
                            nc.sync.dma_start(
                                out_d[
                                    P * m : P * (m + 1),
                                    NQ * n + 256 * oh : NQ * n + 256 * (oh + 1),
                                ],
                                ot[:, 256 * oh : 256 * (oh + 1)],
                            )
                    ops.append(last)
                return ops

            # pair-0 fillers, deadline order: v transposes for jc1-3, then
            # q-proj pair0-n3, then all of pair-1's q projections
            filler_q.append(make_qproj_chunk(0, 0, 3))
            for n in range(4):
                filler_q.append(make_qproj_chunk(1, 128, n))
            # flatten with chunk boundaries preserved (chunks already ordered
            # by deadline; ops within a chunk must stay in order)
            flat_fillers = [op for ch in filler_q for op in ch]
            fill_pos = {"i": 0}

            def emit_fillers(k):
                for _ in range(k):
                    if fill_pos["i"] >= len(flat_fillers):
                        return
                    flat_fillers[fill_pos["i"]]()
                    fill_pos["i"] += 1

            last_scatter = [None, None]
            early_p1_scatter = [None]
            cc_insts = []
            for p in range(2):
                if p == 1:
                    # drain leftover q-proj fillers (they read h), free the
                    # h/wq SBUF, then start streaming w_o into that space
                    emit_fillers(len(flat_fillers) - fill_pos["i"])
                    hq_ctx.__exit__(None, None, None)
                    emit_wo_stream()
                for jc in range(NJC):
                    if p == 1:
                        emit_wo_dmas(jc)
                    nkb = 4 * (jc + 1)
                    ctxs = [
                        ps_ctx.tile([P, NQ], F32, tag=f"ctx{hh}", name=f"ctx{hh}")
                        for hh in range(2)
                    ]
                    sp_prev = None
                    e_prev = None
                    for kb in range(nkb):
                        d = kb - 4 * jc
                        emit_fillers(2 if p == 0 else 3)
                        sp = ps_s.tile([P, 2 * NQ], F32, tag="sp", name="sp")
                        if d >= 0 and USE_MASKMUL:
                            for hh in range(2):
                                base = 64 * hh
                                nc.tensor.matmul(
                                    sp[:, NQ * hh : NQ * (hh + 1)],
                                    kdup[base : base + 64, P * kb : P * (kb + 1)],
                                    qpair[p][base : base + 64, NQ * jc : NQ * (jc + 1)],
                                    start=True,
                                    stop=True,
                                )
                        elif d >= 0:
                            for hh in range(2):
                                co = NQ * hh + 128 * d
                                nc.tensor.matmul(
                                    sp[:, co : co + 128],
                                    identb[:],
                                    tri[:],
                                    start=True,
                                    stop=False,
                                    skip_group_check=True,
                                )
                            for hh in range(2):
                                base = 64 * hh
                                co = NQ * hh + 128 * d
                                nc.tensor.matmul(
                                    sp[:, co : co + 128],
                                    kdup[base : base + 64, P * kb : P * (kb + 1)],
                                    qpair[p][
                                        base : base + 64,
                                        NQ * jc + 128 * d : NQ * jc + 128 * (d + 1),
                                    ],
                                    start=False,
                                    stop=True,
                                    skip_group_check=True,
                                )
                            if d < 3:
                                for hh in range(2):
                                    base = 64 * hh
                                    co = NQ * hh + 128 * (d + 1)
                                    nc.tensor.matmul(
                                        sp[:, co : NQ * (hh + 1)],
                                        kdup[base : base + 64, P * kb : P * (kb + 1)],
                                        qpair[p][
                                            base : base + 64,
                                            NQ * jc + 128 * (d + 1) : NQ * (jc + 1),
                                        ],
                                        start=True,
                                        stop=True,
                                        skip_group_check=True,
                                    )
                        else:
                            for hh in range(2):
                                base = 64 * hh
                                nc.tensor.matmul(
                                    sp[:, NQ * hh : NQ * (hh + 1)],
                                    kdup[base : base + 64, P * kb : P * (kb + 1)],
                                    qpair[p][base : base + 64, NQ * jc : NQ * (jc + 1)],
                                    start=True,
                                    stop=True,
                                )
                        # PV for previous kb (software pipeline depth 1);
                        # diagonal blocks contribute nothing to the columns
                        # left of their stripe, so trim both e and ctx there
                        if kb > 0:
                            off = 128 * (d - 1) if (d >= 1 and not USE_MASKMUL) else 0
                            for hh in range(2):
                                nc.tensor.matmul(
                                    ctxs[hh][:, off:NQ],
                                    v_aug[:, (kb - 1) * 128 : kb * 128],
                                    e_prev[:, NQ * hh + off : NQ * (hh + 1)],
                                    start=(kb - 1 == 0),
                                    stop=False,
                                    skip_group_check=True,
                                )
                        # exp, trimmed to the live columns (per-hh 2D APs)
                        e = esb.tile([P, 2 * NQ], BF16, tag="e", name="e")
                        if USE_MASKMUL:
                            nc.scalar.activation(
                                e[:], sp[:], mybir.ActivationFunctionType.Exp,
                                scale=0.125,
                            )
                            if d >= 0:
                                nc.vector.tensor_mul(
                                    e[:], e[:],
                                    masks[:, 2 * NQ * d : 2 * NQ * (d + 1)],
                                )
                        elif d >= 1:
                            for hh in range(2):
                                nc.scalar.activation(
                                    e[:, NQ * hh + 128 * d : NQ * (hh + 1)],
                                    sp[:, NQ * hh + 128 * d : NQ * (hh + 1)],
                                    mybir.ActivationFunctionType.Exp,
                                    scale=0.125,
                                )
                        elif not USE_MASKMUL:
                            nc.scalar.activation(
                                e[:], sp[:], mybir.ActivationFunctionType.Exp,
                                scale=0.125,
                            )
                        sp_prev, e_prev = sp, e
                    # final PV (always a d=3 diagonal block)
                    foff = 0 if USE_MASKMUL else 384
                    for hh in range(2):
                        nc.tensor.matmul(
                            ctxs[hh][:, foff:NQ],
                            v_aug[:, (nkb - 1) * 128 : nkb * 128],
                            e_prev[:, NQ * hh + foff : NQ * (hh + 1)],
                            start=False,
                            stop=True,
                            skip_group_check=True,
                        )
                    # epilogue: normalize + scatter
                    for hh in range(2):
                        cb = small.tile([P, NQ], F32, tag="cb")
                        nc.vector.tensor_copy(cb[:], ctxs[hh][:, :])
                        # denominators sit replicated in rows 64-127; stage at
                        # base 0 (custom DVE ops misread at nonzero base)
                        rb = small.tile([64, NQ], F32, tag="rb")
                        nc.vector.tensor_copy(rb[:], cb[64:128, :])
                        rec = small.tile([64, NQ], F32, tag="rec")
                        nc.vector.reciprocal_approx_fast(rec[:], rb[:])
                        ctxn = small.tile([64, NQ], BF16, tag="ctxn")
                        nc.vector.tensor_mul(ctxn[:], cb[0:64, :], rec[:])
                        for half in range(2):
                            j = 2 * jc + half
                            last_scatter[p] = nc.sync.dma_start(
                                cc_in[p][
                                    P * j + 64 * hh : P * j + 64 * (hh + 1), :
                                ],
                                ctxn[:, SROWS * half : SROWS * (half + 1)],
                            )
                            if p == 1 and jc == 0:
                                early_p1_scatter[0] = last_scatter[p]
                # A2A for this pair
                cc_insts.append(
                    nc.gpsimd.collective_compute(
                        "AllToAll",
                        mybir.AluOpType.bypass,
                        replica_groups=rg,
                        ins=[cc_in[p].opt()],
                        outs=[cc_out[p].opt()],
                    )
                )

            # ---- tail (baseline structure): drain fillers, then cc_sb loads
            # for both parities; each waits on its own A2A, so even-kt o_proj
            # chunks execute during A2A-1 ----
            emit_fillers(len(flat_fillers) - fill_pos["i"])
            # even loads first: odd loads wait on A2A-1 and would otherwise
            # head-of-line block the even data the A2A-1-covering even o_proj
            # pass needs
            for par in range(2):
                for j in range(NCORES):
                    dma = nc.sync.dma_start(
                        cc_sb[
                            :,
                            (2 * j + par) * SROWS : (2 * j + par) * SROWS + SROWS,
                        ],
                        cc_out[par][j * P : (j + 1) * P, :],
                    )
                    anchor = early_p1_scatter[0] if par == 0 else last_scatter[1]
                    _br.add_dep_helper(
                        dma.ins, anchor.ins, sync=True,
                        reason="cc_sb read ordered behind scatter traffic",
                    )
            for c in range(8):
                for op in make_oproj_chunk(c, 0):
                    op()
            # odd passes + combine + output
            for c in range(8):
                for op in make_oproj_chunk(c, 1):
                    op()

            fil_ctx.__exit__(None, None, None)
            ps_ctx_ctx.__exit__(None, None, None)
            ps_s_ctx.__exit__(None, None, None)
            wo_ctx.__exit__(None, None, None)

    nc.compile()
    return nc


def _get_nc():
    global _NC_CACHE
    if _NC_CACHE is None:
        _NC_CACHE = _build_nc()
    return _NC_CACHE


def _stage_inputs(position_ids, hidden_states, w_qkv, w_o):
    """Host-side sharding / layout staging. Returns in_maps for 8 cores."""
    pos = np.asarray(position_ids)[0].astype(np.float32)            # [S]
    hidden = np.asarray(hidden_states, dtype=np.float32)[0]         # [S, H]
    w_qkv = np.asarray(w_qkv, dtype=np.float32)                     # [3072, H]
    w_o = np.asarray(w_o, dtype=np.float32)                         # [H, Q_SIZE]

    # hT tiles: [H, S] -> [128, KT*S] (k-tile kt at cols kt*S..)
    hT = np.ascontiguousarray(hidden.T)
    hT_r = np.ascontiguousarray(
        hT.reshape(KT, P, S).transpose(1, 0, 2).reshape(P, KT * S)
    ).astype(BF16_NP)

    # w_o.T tiles: [Q_SIZE, H] -> [128, KT*H]
    woT = np.ascontiguousarray(w_o.T)
    woT_r = np.ascontiguousarray(
        woT.reshape(KT, P, H).transpose(1, 0, 2).reshape(P, KT * H)
    ).astype(BF16_NP)

    # rope tables in [d, s] layout for a [128 = 2 heads x 64] tile; the sin
    # product reads its x input at the partner rows, so the table rows hold
    # the sign for the OUTPUT row: [-sin; +sin] per 64-row head block
    inv_freq = (1.0 / (ROPE_THETA ** (np.arange(0, HD, 2, dtype=np.float32) / HD)))
    ang = pos[:, None] * inv_freq[None, :]                          # [S, 32]
    cosT = np.cos(ang).T.astype(np.float32)                         # [32, S]
    sinT = np.sin(ang).T.astype(np.float32)
    ropec = np.concatenate([cosT, cosT, cosT, cosT], axis=0)        # [128, S]
    ropesP = np.concatenate([-sinT, sinT, -sinT, sinT], axis=0)     # [128, S]

    ident = np.eye(P, dtype=np.float32)
    f = np.arange(NQ)
    mask_list = []
    for d in range(4):
        pp = np.arange(P)[:, None] + 128 * d
        mk = (pp <= f[None, :]).astype(BF16_NP)
        mask_list.append(mk)
        mask_list.append(mk)
    masks_np = np.concatenate(mask_list, axis=1)                    # [128, 8*NQ]
    tri = np.where(
        np.arange(P)[:, None] > np.arange(P)[None, :], MASKVAL, 0.0
    ).astype(np.float32)

    in_maps = []
    for i in range(NCORES):
        rows_q = w_qkv[QH * HD * i : QH * HD * (i + 1)]             # [256, H]
        row_k = w_qkv[Q_SIZE + HD * i : Q_SIZE + HD * (i + 1)]      # [64, H]
        row_v = w_qkv[Q_SIZE + KV_SIZE + HD * i : Q_SIZE + KV_SIZE + HD * (i + 1)]
        wshard = np.concatenate([rows_q, row_k, row_v], axis=0)     # [384, H]
        wqkvT = np.ascontiguousarray(wshard.T)                      # [H, 384]
        wqkvT_r = np.ascontiguousarray(
            wqkvT.reshape(KT, P, 384).transpose(1, 0, 2).reshape(P, KT * 384)
        ).astype(BF16_NP)
        in_maps.append(
            {
                "hT": hT_r,
                "wqkvT": wqkvT_r,
                "woT": woT_r,
                "ropec": ropec.astype(BF16_NP),
                "ropesP": ropesP.astype(BF16_NP),
                "identf": ident,
                "identb": ident.astype(BF16_NP),
                "tri": tri.astype(BF16_NP),
                "masks": masks_np,
            }
        )
    return in_maps


def _ensure_ntff_hook():
    """The container's antenv stub lacks axon_hooks, so trn_boot silently
    skipped NTFF hook registration. Recreate the module and register the
    ctypes-based hook so run_bass_kernel_spmd(trace=True) can profile."""
    import sys
    import types

    if "antenv.axon_hooks" in sys.modules:
        return
    try:
        import antenv
        from trn_agent_boot.trn_boot import _ntff_profile_via_ctypes

        hooks = types.ModuleType("antenv.axon_hooks")
        _state = {}

        def set_axon_ntff_profile_hook(h):
            _state["h"] = h

        def get_axon_ntff_profile_hook():
            return _state.get("h")

        hooks.set_axon_ntff_profile_hook = set_axon_ntff_profile_hook
        hooks.get_axon_ntff_profile_hook = get_axon_ntff_profile_hook
        sys.modules["antenv.axon_hooks"] = hooks
        antenv.axon_hooks = hooks
        hook = _ntff_profile_via_ctypes("/opt/axon/libaxon_pjrt.so")
        if hook is not None:
            set_axon_ntff_profile_hook(hook)
    except Exception:
        pass


def kernel(**inputs):
    global LAST_RESULTS
    from concourse.bass_utils import run_bass_kernel_spmd

    nc = _get_nc()
    in_maps = _stage_inputs(
        inputs["position_ids"], inputs["hidden_states"], inputs["w_qkv"], inputs["w_o"]
    )
    trace = os.environ.get("KERNEL_TRACE", "0") == "1"
    if trace:
        _ensure_ntff_hook()
    res = run_bass_kernel_spmd(
        nc, in_maps, core_ids=list(range(NCORES)), trace=trace
    )
    LAST_RESULTS = res
    outs = [np.asarray(res.results[i]["out"], dtype=np.float32) for i in range(NCORES)]
    full = np.concatenate(outs, axis=0)                             # [S, H]
    return full.reshape(1, S, H)
